# revision 1
# baseline (speedup 1.0000x reference)
"""DeepSet baseline kernel for Trainium2 (8 NeuronCores, data-parallel).

Model (reference):
    mask  = sign(|sum_e words|)                  # padding rows are all-zero
    h1    = tanh(words @ W1 + b1)                # [B,S,H]
    h2    = tanh(h1 @ W2 + b2)                   # [B,S,H]
    enc   = h2 @ W3 + b3                         # [B,S,C]
    codes = sum_s enc * mask                     # [B,C]
    out   = (tanh(tanh(codes@W4+b4)@W5+b5)) @ W6 + b6   # [B,T]

Key algebraic restructuring: codes = (sum_s mask*h2) @ W3 + N_b * b3, so the
third big matmul collapses to a [B,H]x[H,C] — only the two big MLP layers run
over all rows.  The device computes hsum[b] = sum_{s valid} h2[b,s,:]; the
tiny decode (<0.01% of FLOPs) runs on host.

Raggedness: valid rows are packed contiguously on host (segment sum is
permutation invariant) and split evenly over the 8 cores; a per-core selection
matrix sel[row, seg] (0/1) encodes both set membership and the validity mask,
applied as a matmul.  Cores run an identical program (SPMD) on different data.

Device pipeline per core (R rows, zero padded):
    a0  = words^T          [E on partitions, rows free]   (host pre-transposed)
    L1: psum[h,r] = sum_e W1[e,h] a0[e,r]; a1 = tanh(psum + b1)   (bias is
        per-partition on ScalarE)                                -> [h, r]
    L2: psum[r,h] = sum_h' a1[h',r] W2[h',h]  (activation tile is the
        stationary operand so the output lands in natural [r,h] layout);
        += b2 broadcast on VectorE; a2 = tanh() on ScalarE       -> [r, h]
    seg: out[t][s,h] = sel[r,s]^T a2[r,h] via matmul accumulated in PSUM per
        row-tile, copied out per tile; host sums the partials.
L2/segment matmuls use float32r (full PE rate at N>=256, ~1e-4 error); L1
optionally runs in bf16 (BF16_L1) to halve the input DMA and enable fast
weight loads.
"""

import sys

if "/opt/trn_rl_repo" not in sys.path:
    sys.path.insert(0, "/opt/trn_rl_repo")

import ml_dtypes
import numpy as np

import concourse.bass as bass
import concourse.mybir as mybir
import concourse.tile as tile
from concourse import bacc
from concourse.bass_utils import run_bass_kernel_spmd

B, S, E = 64, 1024, 512
H = 512
NCORES = 8
P = 128
RT = 512  # rows per row-tile (matmul moving dim)
KC = E // P  # 4 contraction chunks

BF16_L1 = False  # words/W1 in bf16 (L1 only)
BF16_ALL = False  # a1/W2/a2/sel in bf16 too (whole pipeline 16-bit)
N_WARMUP = 12  # dep-free matmuls to open the HAM clock gate during DMA wait

f32 = mybir.dt.float32
f32r = mybir.dt.float32r
bf16 = mybir.dt.bfloat16

_cache: dict = {}


def _tiles_of(R: int):
    """Row-tile sizes: full 512s plus an optional 256 remainder (fp32r needs
    the moving dim >=256 for full PE rate)."""
    assert R % 256 == 0
    return [RT] * (R // RT) + ([256] if R % RT else [])


def _build(R: int, SPAD: int):
    key = (R, SPAD)
    if key in _cache:
        return _cache[key]

    tiles = _tiles_of(R)
    nt = len(tiles)
    offs = [sum(tiles[:i]) for i in range(nt)]
    in_dt = bf16 if BF16_L1 else f32
    l1_dt = bf16 if BF16_L1 else f32r
    act_dt = bf16 if BF16_ALL else f32r
    w2_dt = bf16 if BF16_ALL else f32r

    nc = bacc.Bacc("TRN2", target_bir_lowering=False, debug=False, num_devices=NCORES)

    wT_d = nc.dram_tensor("wT", [P, KC, R], in_dt, kind="ExternalInput").ap()
    sel_d = nc.dram_tensor("sel", [P, R // P, SPAD], f32 if not BF16_ALL else bf16, kind="ExternalInput").ap()
    w1_d = nc.dram_tensor("w1", [E, H], in_dt, kind="ExternalInput").ap()
    w2_d = nc.dram_tensor("w2", [H, H], f32 if not BF16_ALL else bf16, kind="ExternalInput").ap()
    b1_d = nc.dram_tensor("b1", [H], f32, kind="ExternalInput").ap()
    b2b_d = nc.dram_tensor("b2b", [P, H], f32, kind="ExternalInput").ap()
    out_d = nc.dram_tensor("hsum", [nt, SPAD, H], f32, kind="ExternalOutput").ap()

    def cast_l1(ap):
        return ap if BF16_L1 else ap.bitcast(f32r)

    with tile.TileContext(nc) as tc:
        with (
            tc.tile_pool(name="const", bufs=1) as cpool,
            tc.tile_pool(name="a0", bufs=3) as a0pool,
            tc.tile_pool(name="a1", bufs=3) as a1pool,
            tc.tile_pool(name="a2", bufs=6) as a2pool,
            tc.tile_pool(name="ps1", bufs=4, space="PSUM") as ps1pool,
            tc.tile_pool(name="ps2", bufs=2, space="PSUM") as ps2pool,
            tc.tile_pool(name="ps3", bufs=2, space="PSUM") as ps3pool,
        ):
            # PE warmup: dependency-free bf16 matmuls issued first so the HAM
            # clock-gate opens (1.2 -> 2.4 GHz) while the first DMAs land.
            warm_sb = cpool.tile([P, RT], bf16)
            nc.gpsimd.memset(warm_sb[:], 0.25)
            for w in range(N_WARMUP):
                wps = ps1pool.tile([P, RT], f32, tag="ps1", name="wps")
                nc.tensor.matmul(
                    wps[:, :256], warm_sb[:, :P], warm_sb[:, :256],
                    start=True, stop=True,
                )

            # DMA issue order = critical path first, at k-chunk granularity:
            # the first L1 matmul only needs w1[k=0] + a0[t=0][k=0].
            w1k = []
            w2k = []
            a0_pre = {0: [], 1: []}
            for k in range(KC):
                w1c = cpool.tile([P, H], l1_dt, name=f"w1k{k}")
                nc.sync.dma_start(w1c[:], cast_l1(w1_d[k * P:(k + 1) * P, :]))
                w1k.append(w1c)
                a0c = a0pool.tile([P, RT], l1_dt, tag=f"a0k{k}", name=f"a0k{k}")
                nc.sync.dma_start(
                    a0c[:, :tiles[0]],
                    cast_l1(wT_d[:, k, offs[0]:offs[0] + tiles[0]]),
                )
                a0_pre[0].append(a0c)
            b1sb = cpool.tile([P, KC], f32)
            nc.sync.dma_start(b1sb[:], b1_d.rearrange("(hc p) -> p hc", p=P))
            for k in range(KC):
                w2c = cpool.tile([P, H], w2_dt, name=f"w2k{k}")
                if BF16_ALL:
                    nc.sync.dma_start(w2c[:], w2_d[k * P:(k + 1) * P, :])
                else:
                    nc.sync.dma_start(
                        w2c[:], w2_d[k * P:(k + 1) * P, :].bitcast(f32r)
                    )
                w2k.append(w2c)
            if nt > 1:
                for k in range(KC):
                    a0c = a0pool.tile([P, RT], l1_dt, tag=f"a0k{k}", name=f"a0k{k}")
                    nc.sync.dma_start(
                        a0c[:, :tiles[1]],
                        cast_l1(wT_d[:, k, offs[1]:offs[1] + tiles[1]]),
                    )
                    a0_pre[1].append(a0c)
            selsb = cpool.tile([P, R // P, SPAD], act_dt)
            if BF16_ALL:
                nc.sync.dma_start(selsb[:], sel_d)
            else:
                nc.sync.dma_start(selsb[:], sel_d.bitcast(f32r))
            b2sb = cpool.tile([P, H], f32)
            nc.sync.dma_start(b2sb[:], b2b_d)

            for t in range(nt):
                nr = tiles[t]
                nsub = nr // P
                if t in a0_pre and a0_pre[t]:
                    a0 = a0_pre.pop(t)
                else:
                    a0 = []
                    for k in range(KC):
                        a0c = a0pool.tile(
                            [P, RT], l1_dt, tag=f"a0k{k}", name=f"a0k{k}"
                        )
                        nc.sync.dma_start(
                            a0c[:, :nr],
                            cast_l1(wT_d[:, k, offs[t]:offs[t] + nr]),
                        )
                        a0.append(a0c)
                # --- L1: transposed output [h, r] ---
                # tile 0 runs k-outer so each arriving (w1,a0) chunk pair
                # feeds 4 matmuls — matches the DMA delivery rate at startup
                a1 = [
                    a1pool.tile([P, RT], act_dt, tag=f"a1c{m}", name=f"a1c{m}")
                    for m in range(KC)
                ]
                if t == 0:
                    pss = [
                        ps1pool.tile([P, RT], f32, tag="ps1", name=f"ps1_{m}")
                        for m in range(KC)
                    ]
                    for k in range(KC):
                        for m in range(KC):
                            nc.tensor.matmul(
                                pss[m][:, :nr],
                                w1k[k][:, m * P:(m + 1) * P],
                                a0[k][:, :nr],
                                start=(k == 0),
                                stop=(k == KC - 1),
                            )
                    for m in range(KC):
                        nc.scalar.activation(
                            a1[m][:, :nr],
                            pss[m][:, :nr],
                            mybir.ActivationFunctionType.Tanh,
                            bias=b1sb[:, m:m + 1],
                        )
                else:
                    for m in range(KC):
                        ps = ps1pool.tile([P, RT], f32, tag="ps1")
                        for k in range(KC):
                            nc.tensor.matmul(
                                ps[:, :nr],
                                w1k[k][:, m * P:(m + 1) * P],
                                a0[k][:, :nr],
                                start=(k == 0),
                                stop=(k == KC - 1),
                            )
                        nc.scalar.activation(
                            a1[m][:, :nr],
                            ps[:, :nr],
                            mybir.ActivationFunctionType.Tanh,
                            bias=b1sb[:, m:m + 1],
                        )
                # --- L2: natural output [r, h], 128-row subtiles ---
                a2s = []
                for rs in range(nsub):
                    ps2 = ps2pool.tile([P, H], f32, tag="ps2")
                    for k in range(KC):
                        nc.tensor.matmul(
                            ps2[:],
                            a1[k][:, rs * P:(rs + 1) * P],
                            w2k[k][:],
                            start=(k == 0),
                            stop=(k == KC - 1),
                        )
                    nc.vector.tensor_add(ps2[:], ps2[:], b2sb[:])
                    a2 = a2pool.tile([P, H], act_dt, tag="a2")
                    nc.scalar.activation(
                        a2[:], ps2[:], mybir.ActivationFunctionType.Tanh
                    )
                    a2s.append(a2)
                # --- segment sum partial: out[t] = sel^T @ a2 (host sums) ---
                ps3 = ps3pool.tile([SPAD, H], f32, tag="ps3")
                for rs in range(nsub):
                    nc.tensor.matmul(
                        ps3[:],
                        selsb[:, offs[t] // P + rs, :],
                        a2s[rs][:],
                        start=(rs == 0),
                        stop=(rs == nsub - 1),
                    )
                seg_out = a2pool.tile([SPAD, H], f32, tag="segout", name="seg_out")
                nc.vector.tensor_copy(seg_out[:], ps3[:])
                nc.sync.dma_start(out_d[t], seg_out[:])

    nc.compile()
    _cache[key] = nc
    return nc


def _pack(words: np.ndarray):
    """Pack valid rows contiguously, split across cores.

    Returns per-core arrays + bookkeeping to scatter partial segment sums back
    to global set ids.
    """
    words = np.asarray(words, dtype=np.float32)
    mask = np.sign(np.abs(words.sum(axis=-1)))  # [B, S], matches reference
    valid = mask > 0

    rows = []
    segs = []
    for b in range(B):
        vb = words[b][valid[b]]
        rows.append(vb)
        segs.append(np.full(len(vb), b, dtype=np.int64))
    rows = np.concatenate(rows, axis=0)
    segs = np.concatenate(segs, axis=0)
    total = len(rows)

    quota = -(-total // NCORES)  # ceil
    R = -(-quota // 256) * 256  # pad to tile granularity
    cores = []
    spad_needed = 1
    for c in range(NCORES):
        lo, hi = c * quota, min((c + 1) * quota, total)
        chunk = rows[lo:hi]
        seg_chunk = segs[lo:hi]
        n = hi - lo
        if n < R:
            chunk = np.concatenate(
                [chunk, np.zeros((R - n, E), dtype=np.float32)], axis=0
            )
        gids = []
        col_of = {}
        cols = np.zeros(n, dtype=np.int64)
        for i, g in enumerate(seg_chunk):
            if g not in col_of:
                col_of[g] = len(gids)
                gids.append(int(g))
            cols[i] = col_of[g]
        spad_needed = max(spad_needed, len(gids))
        cores.append((chunk, cols, n, gids))

    SPAD = max(8, -(-spad_needed // 8) * 8)
    assert SPAD <= P, f"too many segments per core: {spad_needed}"

    in_np = ml_dtypes.bfloat16 if BF16_L1 else np.float32
    per_core = []
    for chunk, cols, n, gids in cores:
        wT = np.ascontiguousarray(
            chunk.T.reshape(KC, P, R).transpose(1, 0, 2)
        ).astype(in_np)  # [P, KC, R]
        sel = np.zeros((R, SPAD), dtype=np.float32)
        if n:
            sel[np.arange(n), cols] = 1.0
        sel = np.ascontiguousarray(
            sel.reshape(R // P, P, SPAD).transpose(1, 0, 2)
        )  # [P, R//P, SPAD] — matches the SBUF tile layout exactly
        if BF16_ALL:
            sel = sel.astype(ml_dtypes.bfloat16)
        per_core.append((wT, sel, gids))
    return per_core, R, SPAD, mask


def _in_maps(per_core, inputs):
    W1 = np.asarray(inputs["W1"], dtype=ml_dtypes.bfloat16 if BF16_L1 else np.float32)
    W2 = np.asarray(inputs["W2"], dtype=ml_dtypes.bfloat16 if BF16_ALL else np.float32)
    b1 = np.asarray(inputs["b1"], dtype=np.float32)
    b2 = np.asarray(inputs["b2"], dtype=np.float32)
    b2b = np.broadcast_to(b2[None, :], (P, H)).copy()
    return [
        {"wT": wT, "sel": sel, "w1": W1, "w2": W2, "b1": b1, "b2b": b2b}
        for (wT, sel, _g) in per_core
    ]


def kernel(words, W1, b1, W2, b2, W3, b3, W4, b4, W5, b5, W6, b6):
    per_core, R, SPAD, mask = _pack(words)
    nc = _build(R, SPAD)
    in_maps = _in_maps(
        per_core, {"W1": W1, "W2": W2, "b1": b1, "b2": b2}
    )

    res = run_bass_kernel_spmd(nc, in_maps, core_ids=list(range(NCORES)))

    hsum = np.zeros((B, H), dtype=np.float32)
    for c in range(NCORES):
        out_c = res.results[c]["hsum"].sum(axis=0)
        for j, g in enumerate(per_core[c][2]):
            hsum[g] += out_c[j]

    # host decode (tiny)
    lengths = mask.sum(axis=1).astype(np.float32)[:, None]
    codes = hsum @ np.asarray(W3, np.float32) + lengths * np.asarray(b3, np.float32)
    h = np.tanh(codes @ np.asarray(W4, np.float32) + np.asarray(b4, np.float32))
    h = np.tanh(h @ np.asarray(W5, np.float32) + np.asarray(b5, np.float32))
    out = h @ np.asarray(W6, np.float32) + np.asarray(b6, np.float32)
    return out.astype(np.float32)



# revision 2
# speedup vs baseline: 1.2190x; 1.2190x over previous
"""DeepSet kernel for Trainium2 (8 NeuronCores, data-parallel).

Model (reference):
    mask  = sign(|sum_e words|)                  # padding rows are all-zero
    h1    = tanh(words @ W1 + b1)                # [B,S,H]
    h2    = tanh(h1 @ W2 + b2)                   # [B,S,H]
    enc   = h2 @ W3 + b3                         # [B,S,C]
    codes = sum_s enc * mask                     # [B,C]
    out   = (tanh(tanh(codes@W4+b4)@W5+b5)) @ W6 + b6   # [B,T]

Algebraic restructuring: codes = (sum_s mask*h2) @ W3 + N_b * b3, so only the
two big MLP layers run on device; the tiny decode runs on host.

Layout strategy (all bf16 on the PE, fp32 psum):
  - valid rows packed contiguously, G=32-aligned per set: every set's rows are
    padded with zero-rows to a multiple of G so that every G-row block belongs
    to exactly one set.  Blocks are dealt to 8 cores (SPMD, identical
    programs).  A zero pad row produces the CONSTANT vector
    g = tanh(tanh(b1)@W2+b2) after the two layers; the host subtracts
    n_pad(set) * g, so no selection mask is needed on device.
  - L1: a0 = words^T [e on partitions, rows free]; ps1[h,r] accumulated over
    4 e-chunks; a1 = tanh(ps1 + b1) via per-partition activation bias.
  - L2 TRANSPOSED: ps2[h,r] = sum_h' W2[h',h] a1[h',r] keeps h on partitions,
    so b2 also rides the activation bias (no vector add) and the segment sum
    is a free-dim reduction: VectorE block-reduces a2[h, r] in G-row blocks
    -> acc[h, block].  Host maps blocks to sets.
  - PE does ONLY the two 512x512 GEMMs: 32*R cycles/core @2.4GHz.
  - Startup: DVE memsets a warmup tile early; ~16 dependency-free matmuls keep
    the PE busy from ~5us so the HAM clock gate (4/8 -> 8/8 duty) opens before
    the real data lands; DMAs are issued critical-path-first.
"""

import sys

if "/opt/trn_rl_repo" not in sys.path:
    sys.path.insert(0, "/opt/trn_rl_repo")

import ml_dtypes
import numpy as np

import concourse.bass as bass
import concourse.mybir as mybir
import concourse.tile as tile
from concourse import bacc
from concourse.bass_utils import run_bass_kernel_spmd

B, S, E = 64, 1024, 512
H = 512
NCORES = 8
P = 128
KC = E // P  # 4 contraction chunks
RT = 512     # rows per row-tile (matmul moving dim)
G = 32       # segment alignment granularity (block reduce size)
NBT = RT // G  # blocks per full row tile
N_WARMUP = 16  # dep-free matmuls to open the HAM clock gate during DMA wait

f32 = mybir.dt.float32
bf16 = mybir.dt.bfloat16

_cache: dict = {}


def _tiles_of(R: int):
    assert R % G == 0
    tl = [RT] * (R // RT)
    if R % RT:
        tl.append(R % RT)
    return tl


def _build(R: int):
    if R in _cache:
        return _cache[R]

    tiles = _tiles_of(R)
    nt = len(tiles)
    offs = [sum(tiles[:i]) for i in range(nt)]

    nc = bacc.Bacc("TRN2", target_bir_lowering=False, debug=False, num_devices=NCORES)

    wT_d = nc.dram_tensor("wT", [P, KC, R], bf16, kind="ExternalInput").ap()
    w1_d = nc.dram_tensor("w1", [E, H], bf16, kind="ExternalInput").ap()
    w2_d = nc.dram_tensor("w2", [H, H], bf16, kind="ExternalInput").ap()
    b1_d = nc.dram_tensor("b1c", [H], f32, kind="ExternalInput").ap()
    b2_d = nc.dram_tensor("b2c", [H], f32, kind="ExternalInput").ap()
    acc_d = nc.dram_tensor("acc", [nt, P, KC, NBT], f32, kind="ExternalOutput").ap()

    with tile.TileContext(nc) as tc:
        with (
            tc.tile_pool(name="const", bufs=1) as cpool,
            tc.tile_pool(name="a0", bufs=3) as a0pool,
            tc.tile_pool(name="a1", bufs=2) as a1pool,
            tc.tile_pool(name="a2", bufs=2) as a2pool,
            tc.tile_pool(name="accp", bufs=3) as accpool,
            tc.tile_pool(name="ps1", bufs=4, space="PSUM") as ps1pool,
            tc.tile_pool(name="ps2", bufs=3, space="PSUM") as ps2pool,
        ):
            # PE warmup: DVE memsets the tile early (vector's iram load ends
            # ~4.9us); dependency-free bf16 matmuls keep the PE busy so the
            # HAM clock gate (4/8 duty default) opens before real data lands.
            warm_sb = cpool.tile([P, 256], bf16)
            nc.vector.memset(warm_sb[:], 0.25)
            for w in range(N_WARMUP):
                wps = ps1pool.tile([P, RT], f32, tag="ps1", name="wps")
                nc.tensor.matmul(
                    wps[:, :256], warm_sb[:, :P], warm_sb[:, :256],
                    start=True, stop=True,
                )

            # --- DMA issue order = critical path first (all on sync/SP) ---
            # First L1 matmuls (tile0, k-outer) need only w1[k] + a0[t0][k].
            w1sb = cpool.tile([P, KC, H], bf16)
            w1r = w1_d.rearrange("(k p) h -> p k h", p=P)
            nc.sync.dma_start(w1sb[:, 0, :], w1r[:, 0, :])
            a0_pre: dict = {0: [], 1: []}
            a0c = a0pool.tile([P, KC, RT], bf16, tag="a0", name="a0t0")
            nc.sync.dma_start(a0c[:, 0, :tiles[0]], wT_d[:, 0, 0:tiles[0]])
            nc.sync.dma_start(w1sb[:, 1:, :], w1r[:, 1:, :])
            for k in range(1, KC):
                nc.sync.dma_start(
                    a0c[:, k, :tiles[0]], wT_d[:, k, 0:tiles[0]]
                )
            a0_pre[0] = a0c
            b1sb = cpool.tile([P, KC], f32)
            nc.sync.dma_start(b1sb[:], b1_d.rearrange("(m p) -> p m", p=P))
            b2sb = cpool.tile([P, KC], f32)
            nc.sync.dma_start(b2sb[:], b2_d.rearrange("(m p) -> p m", p=P))
            if nt > 1:
                a0c = a0pool.tile([P, KC, RT], bf16, tag="a0", name="a0t1")
                for k in range(KC):
                    nc.sync.dma_start(
                        a0c[:, k, :tiles[1]],
                        wT_d[:, k, offs[1]:offs[1] + tiles[1]],
                    )
                a0_pre[1] = a0c
            w2sb = cpool.tile([P, KC, H], bf16)
            nc.sync.dma_start(w2sb[:], w2_d.rearrange("(k p) h -> p k h", p=P))

            for t in range(nt):
                nr = tiles[t]
                nb = nr // G
                if t in a0_pre:
                    a0 = a0_pre.pop(t)
                else:
                    a0 = a0pool.tile([P, KC, RT], bf16, tag="a0", name=f"a0t{t}")
                    nc.sync.dma_start(
                        a0[:, :, :nr], wT_d[:, :, offs[t]:offs[t] + nr]
                    )
                # --- L1: transposed output [h, r] ---
                a1 = [
                    a1pool.tile([P, RT], bf16, tag=f"a1c{m}", name=f"a1c{m}")
                    for m in range(KC)
                ]
                if t == 0:
                    # k-outer so each arriving a0/w1 chunk feeds 4 matmuls,
                    # matching DMA delivery at startup
                    pss = [
                        ps1pool.tile([P, RT], f32, tag="ps1", name=f"ps1_{m}")
                        for m in range(KC)
                    ]
                    for k in range(KC):
                        for m in range(KC):
                            nc.tensor.matmul(
                                pss[m][:, :nr],
                                w1sb[:, k, m * P:(m + 1) * P],
                                a0[:, k, :nr],
                                start=(k == 0),
                                stop=(k == KC - 1),
                            )
                    for m in range(KC):
                        nc.scalar.activation(
                            a1[m][:, :nr],
                            pss[m][:, :nr],
                            mybir.ActivationFunctionType.Tanh,
                            bias=b1sb[:, m:m + 1],
                        )
                else:
                    for m in range(KC):
                        ps = ps1pool.tile([P, RT], f32, tag="ps1")
                        for k in range(KC):
                            nc.tensor.matmul(
                                ps[:, :nr],
                                w1sb[:, k, m * P:(m + 1) * P],
                                a0[:, k, :nr],
                                start=(k == 0),
                                stop=(k == KC - 1),
                            )
                        nc.scalar.activation(
                            a1[m][:, :nr],
                            ps[:, :nr],
                            mybir.ActivationFunctionType.Tanh,
                            bias=b1sb[:, m:m + 1],
                        )
                # --- L2 transposed: ps2[h, r]; k-inner so the first matmul
                # needs only a1[0] (tanh of later chunks still in flight) ---
                acc = accpool.tile([P, KC, NBT], f32, tag="acc", name="acc")
                for m2 in range(KC):
                    ps2 = ps2pool.tile([P, RT], f32, tag="ps2")
                    for k in range(KC):
                        nc.tensor.matmul(
                            ps2[:, :nr],
                            w2sb[:, k, m2 * P:(m2 + 1) * P],
                            a1[k][:, :nr],
                            start=(k == 0),
                            stop=(k == KC - 1),
                        )
                    a2 = a2pool.tile([P, RT], bf16, tag=f"a2c{m2}")
                    nc.scalar.activation(
                        a2[:, :nr],
                        ps2[:, :nr],
                        mybir.ActivationFunctionType.Tanh,
                        bias=b2sb[:, m2:m2 + 1],
                    )
                    # segment block sums: [h, nb, G] -> [h, nb] on VectorE
                    nc.vector.tensor_reduce(
                        acc[:, m2, :nb],
                        a2[:, :nr].rearrange("p (n g) -> p n g", g=G),
                        mybir.AxisListType.X,
                        mybir.AluOpType.add,
                    )
                nc.sync.dma_start(acc_d[t][:, :, :nb], acc[:, :, :nb])

    nc.compile()
    _cache[R] = nc
    return nc


def _pack(words: np.ndarray):
    """Pack valid rows contiguously, G-aligned per set, dealt to 8 cores.

    Returns per-core bf16 wT arrays + global block bookkeeping.
    """
    words = np.asarray(words, dtype=np.float32)
    mask = np.sign(np.abs(words.sum(axis=-1)))  # [B, S], matches reference
    valid = mask > 0
    lengths = valid.sum(axis=1)

    nblk = -(-lengths // G)  # ceil: blocks per set
    total_blocks = int(nblk.sum())
    pcb = -(-total_blocks // NCORES)  # blocks per core
    R = pcb * G

    rows = np.zeros((NCORES * R, E), dtype=np.float32)
    binfo = np.full(NCORES * pcb, -1, dtype=np.int64)  # set id per block
    off = 0
    for b in range(B):
        vb = words[b][valid[b]]
        L = len(vb)
        rows[off:off + L] = vb
        b0 = off // G
        binfo[b0:b0 + nblk[b]] = b
        off += int(nblk[b]) * G

    per_core = []
    for c in range(NCORES):
        chunk = rows[c * R:(c + 1) * R]
        wT = np.ascontiguousarray(
            chunk.T.reshape(KC, P, R).transpose(1, 0, 2)
        ).astype(ml_dtypes.bfloat16)  # [P, KC, R]
        per_core.append(wT)
    return per_core, R, binfo, mask, lengths


def _in_maps(per_core, inputs):
    W1 = np.asarray(inputs["W1"], dtype=ml_dtypes.bfloat16)
    W2 = np.asarray(inputs["W2"], dtype=ml_dtypes.bfloat16)
    b1 = np.asarray(inputs["b1"], dtype=np.float32)
    b2 = np.asarray(inputs["b2"], dtype=np.float32)
    return [
        {"wT": wT, "w1": W1, "w2": W2, "b1c": b1, "b2c": b2}
        for wT in per_core
    ]


def kernel(words, W1, b1, W2, b2, W3, b3, W4, b4, W5, b5, W6, b6):
    per_core, R, binfo, mask, lengths = _pack(words)
    nc = _build(R)
    in_maps = _in_maps(per_core, {"W1": W1, "W2": W2, "b1": b1, "b2": b2})

    res = run_bass_kernel_spmd(nc, in_maps, core_ids=list(range(NCORES)))

    tiles = _tiles_of(R)
    nt = len(tiles)
    pcb = R // G
    hsum = np.zeros((B, H), dtype=np.float32)
    for c in range(NCORES):
        acc = res.results[c]["acc"]  # [nt, P, KC, NBT] f32
        # block vectors in h order (h = m*128 + p)
        bv = np.concatenate(
            [acc[t][:, :, :tiles[t] // G].transpose(2, 1, 0).reshape(-1, H)
             for t in range(nt)], axis=0
        )  # [pcb, H]
        ids = binfo[c * pcb:(c + 1) * pcb]
        sel = ids >= 0
        np.add.at(hsum, ids[sel], bv[sel])

    # exact correction for zero-pad rows: each contributes the constant
    # g = tanh(tanh(b1) @ W2 + b2) (computed with the same bf16 rounding
    # the device uses, in fp32 accumulation)
    b1f = np.asarray(b1, np.float32)
    b2f = np.asarray(b2, np.float32)
    W2q = np.asarray(W2, np.float32).astype(ml_dtypes.bfloat16).astype(np.float32)
    h1g = np.tanh(b1f).astype(ml_dtypes.bfloat16).astype(np.float32)
    g = np.tanh(h1g @ W2q + b2f)
    npad = (-(-lengths // G) * G - lengths).astype(np.float32)  # per set
    hsum -= npad[:, None] * g[None, :]

    # host decode (tiny)
    codes = hsum @ np.asarray(W3, np.float32) + (
        lengths.astype(np.float32)[:, None] * np.asarray(b3, np.float32)
    )
    h = np.tanh(codes @ np.asarray(W4, np.float32) + np.asarray(b4, np.float32))
    h = np.tanh(h @ np.asarray(W5, np.float32) + np.asarray(b5, np.float32))
    out = h @ np.asarray(W6, np.float32) + np.asarray(b6, np.float32)
    return out.astype(np.float32)
